# revision 4
# baseline (speedup 1.0000x reference)
"""Trainium2 Bass kernel for nn_BasicRNN (2-layer LSTM, H=32, S=64, B=8192).

Strategy: pure data parallel over 8 cores (1024 batch each). Per core the
batch is laid out in "T-layout" tiles [128 partitions = 4 groups x 32
features, 256 batch (free)]. The 256-batch free dim is split into two
128-wide chunks (A/B) whose dependency chains software-pipeline across the
engines.

The wall-clock is bound by the 128-step serial recurrence (64 steps x 2
layers); per step the critical cycle is
    rec-matmuls -> tanh(gates) -> u,v -> c* -> tanh(c*) -> h -> rec-matmuls
so v2 shortens that cycle:
  - input projections for step k+1 are emitted EARLY into their own PSUM
    bank (psA bufs=4) so they never sit on the critical path,
  - the hidden state is kept SPLIT as h* = tc + q with tc = tanh(c*/2)
    and q = to * tc: the 4 tc-side recurrent matmuls issue right after the
    scalar-engine tanh (skipping the h-combine), and q follows via a fast
    2x-mode tensor_tensor multiply; both matmul groups accumulate into the
    same PSUM gates tile (identical weights),
  - layer 1 additionally materializes h1* = (to+1)*tc off-chain (DVE) for
    layer 2's input projections; layer 2's split output feeds the final
    projection as y = WOUT*tc2 + WOUT*q2.
Sigmoids use the tanh identity sigmoid(x) = (1 + tanh(x/2))/2 with scale
factors folded into host-prepped weights; the cell state is stored doubled
(c* = 2c):
    u  = (tanh_f + 1) * c*          v = (tanh_i + 1) * tanh_g
    c* = 0.5*u + v                  tc = tanh(0.5 * c*)
    q  = tanh_o * tc                h* = tc + q  (= (tanh_o + 1) * tc)
"""
import sys
sys.path.insert(0, '/opt/trn_rl_repo')

import numpy as np

import concourse.bacc as bacc
import concourse.tile as tile
from concourse import mybir
from concourse.bass_utils import run_bass_kernel_spmd

F32 = mybir.dt.float32
F16 = mybir.dt.float16
TANH = mybir.ActivationFunctionType.Tanh
IDENT = mybir.ActivationFunctionType.Identity
ADD = mybir.AluOpType.add
MULT = mybir.AluOpType.mult

B, S, NX, NSFC, H, NY = 8192, 64, 4, 5, 32, 1
NCORES = 8
BC = B // NCORES          # 1024 batch per core
NG = 4                    # groups per core (partition strips)
GB = BC // NG             # 256 batch per group
CB = GB // 2              # 128 batch per chunk (A/B split of the free dim)
# gate order in the G-tile free dim: (g, f, i, o), 128 cols each.
GATES = [("g", 2 * H, 1.0), ("f", H, 0.5), ("i", 0, 0.5), ("o", 3 * H, 0.5)]

_CACHED = {}


def _prep_weights(inp):
    """Host-side weight staging. Returns dict of DRAM arrays (shared by all
    cores)."""
    w = {}

    def blockdiag(wmat, scale_fio, scale_g):
        # [128, 512]: col block gi = block-diag lhsT for gate gi.
        t = np.zeros((128, 512), np.float32)
        for gi, (nm, r0, _) in enumerate(GATES):
            s = scale_g if nm == "g" else scale_fio
            blk = (wmat[r0:r0 + H] * s).T.astype(np.float32)  # [K=H, 32]
            for g in range(NG):
                t[32 * g:32 * g + H,
                  128 * gi + 32 * g:128 * gi + 32 * g + 32] = blk
        return t.astype(np.float16)

    # layer-1 input lhsT [20, 512]: rows 5g+q (q<4 -> x features, q=4 ->
    # bias row). x is true scale; output h1* doubled elsewhere.
    t = np.zeros((20, 512), np.float32)
    btot1 = inp["b_ih1"] + inp["b_hh1"]
    for gi, (nm, r0, trick) in enumerate(GATES):
        s = 1.0 if nm == "g" else 0.5
        blk = (inp["w_ih1"][r0:r0 + H] * s).T.astype(np.float32)  # [NX, 32]
        for g in range(NG):
            c0 = 128 * gi + 32 * g
            t[5 * g:5 * g + NX, c0:c0 + 32] = blk
            t[5 * g + NX, c0:c0 + 32] = btot1[r0:r0 + H] * trick
    w["WX1"] = t.astype(np.float16)

    w["WL1"] = blockdiag(inp["w_hh1"], 0.25, 0.5)
    w["WX2"] = blockdiag(inp["w_ih2"], 0.25, 0.5)   # input h1* is doubled
    w["WL2"] = blockdiag(inp["w_hh2"], 0.25, 0.5)

    # layer-2 bias matmul: lhsT B2 [128, 128]: row 32g+a holds gate-a bias
    # values at cols 32g+j; rhs ONESB [128, 512]: row 32g+a is 1.0 exactly
    # on gate-a's 128-col block.
    b2 = np.zeros((128, 128), np.float32)
    onesb = np.zeros((128, 512), np.float32)
    btot2 = inp["b_ih2"] + inp["b_hh2"]
    for gi, (_, r0, trick) in enumerate(GATES):
        for g in range(NG):
            b2[32 * g + gi, 32 * g:32 * g + 32] = btot2[r0:r0 + H] * trick
            onesb[32 * g + gi, 128 * gi:128 * (gi + 1)] = 1.0
    w["B2"] = b2.astype(np.float16)
    w["ONESB"] = onesb.astype(np.float16)

    # sfc weights [8, 64]: rows 0:5 = [w_sfc1.T | w_sfc2.T]
    ws = np.zeros((8, 64), np.float32)
    ws[:NSFC, 0:32] = inp["w_sfc1"].T
    ws[:NSFC, 32:64] = inp["w_sfc2"].T
    w["WSFC"] = ws
    bs = np.zeros((128, 2), np.float32)
    for g in range(NG):
        bs[32 * g:32 * g + 32, 0] = inp["b_sfc1"]
        bs[32 * g:32 * g + 32, 1] = inp["b_sfc2"]
    w["BSFC"] = bs

    # output weights [128, 1]: block g = (w_out * 0.5).T
    wo = np.zeros((128, 1), np.float32)
    for g in range(NG):
        wo[32 * g:32 * g + 32, 0] = inp["w_out"][0] * 0.5
    w["WOUT"] = wo.astype(np.float16)
    w["BOUT"] = np.full((128, 1), float(inp["b_out"][0]), np.float32)
    return w


def build_program(n_steps=S, trace_sim=False, reps=0, split_o=False):
    nc = bacc.Bacc()
    d = {}
    d["xs"] = nc.declare_dram_parameter("xs", [n_steps, 20, GB], F16,
                                        isOutput=False)
    d["sfcT"] = nc.declare_dram_parameter("sfcT", [8, BC], F32, isOutput=False)
    F16W = {"WX1", "WL1", "WX2", "WL2", "B2", "ONESB", "WOUT"}
    WSHAPES = [("WX1", [20, 512]), ("WL1", [128, 512]),
               ("WX2", [128, 512]), ("WL2", [128, 512]),
               ("B2", [128, 128]), ("ONESB", [128, 512]),
               ("WSFC", [8, 64]), ("BSFC", [128, 2]),
               ("WOUT", [128, 1]), ("BOUT", [128, 1])]
    for nm, shape in WSHAPES:
        d[nm] = nc.declare_dram_parameter(nm, shape,
                                          F16 if nm in F16W else F32,
                                          isOutput=False)
    y_out = nc.declare_dram_parameter("y", [NG, n_steps * GB], F32,
                                      isOutput=True)

    NS1 = n_steps + 1

    with tile.TileContext(nc, trace_sim=trace_sim) as tc:
        with tc.tile_pool(name="wpool", bufs=1) as wpool, \
             tc.tile_pool(name="big", bufs=1) as big, \
             tc.tile_pool(name="work", bufs=3) as work, \
             tc.tile_pool(name="xp", bufs=3) as xp, \
             tc.tile_pool(name="yp", bufs=2) as yp, \
             tc.tile_pool(name="psA", bufs=3, space="PSUM") as psA, \
             tc.tile_pool(name="psY", bufs=1, space="PSUM") as psY:

            # ---- stage weights ----
            W = {}
            for nm, shape in WSHAPES:
                t = wpool.tile(shape, F16 if nm in F16W else F32, tag=nm)
                nc.sync.dma_start(t[:], d[nm][:])
                W[nm] = t
            sfcT = wpool.tile([8, BC], F32, tag="sfcT")
            nc.sync.dma_start(sfcT[:], d["sfcT"][:])

            # ---- big state storage (all fp16) ----
            # tc/q split hidden state per layer; h1 materialized for layer-2
            # input projections.
            tc1 = big.tile([128, NS1 * GB], F16, tag="tc1")   # slot n_steps = h0*
            q1 = big.tile([128, n_steps * GB], F16, tag="q1")
            h1 = big.tile([128, n_steps * GB], F16, tag="h1")
            tc2 = big.tile([128, NS1 * GB], F16, tag="tc2")   # slot 0 = 0
            q2 = big.tile([128, NS1 * GB], F16, tag="q2")     # slot 0 unused

            # CT tiles [128, 5*CB] fp16: [c*slot | tg | tf | ti | to]
            def new_CT(ch):
                return work.tile([128, 5 * CB], F16, tag=f"CT{ch}",
                                 name=f"CT{ch}")

            # ---- init: h0/c0 from surface MLPs ----
            ph = psA.tile([128, 2 * GB], F32, tag="G0")
            for g in range(NG):
                nc.tensor.matmul(ph[32 * g:32 * g + 32, 0:GB],
                                 W["WSFC"][0:NSFC, 0:32],
                                 sfcT[0:NSFC, GB * g:GB * (g + 1)],
                                 start=True, stop=True,
                                 tile_position=(0, 32 * g))
                nc.tensor.matmul(ph[32 * g:32 * g + 32, GB:2 * GB],
                                 W["WSFC"][0:NSFC, 32:64],
                                 sfcT[0:NSFC, GB * g:GB * (g + 1)],
                                 start=True, stop=True,
                                 tile_position=(0, 32 * g))
            t0 = work.tile([128, GB], F32, tag="t0")
            nc.scalar.activation(t0[:], ph[:, 0:GB], TANH, bias=W["BSFC"][:, 0:1])
            # h0* = 2*tanh(...)  stored in tc1 slot n_steps (q-half is zero
            # and simply skipped at k==0)
            nc.vector.tensor_scalar_mul(
                tc1[:, n_steps * GB:(n_steps + 1) * GB], t0[:], 2.0)
            t0b = work.tile([128, GB], F32, tag="t0")
            nc.scalar.activation(t0b[:], ph[:, GB:2 * GB], TANH,
                                 bias=W["BSFC"][:, 1:2])
            # layer-2 zero init state
            nc.vector.memset(tc2[:, 0:GB], 0.0)

            # ---- the two sequential LSTM layers ----
            def emit_inputs(layer, k, ch, xstep, final_stop=False):
                """Start a fresh PSUM gates tile for (k, ch): bias + input
                projections (start group; stop only if no rec matmuls will
                follow)."""
                G = psA.tile([128, 4 * CB], F32, tag=f"G{ch}")
                if layer == 1:
                    for gi in range(4):
                        nc.tensor.matmul(
                            G[:, gi * CB:(gi + 1) * CB],
                            W["WX1"][0:20, gi * 128:(gi + 1) * 128],
                            xstep[0:20, ch * CB:(ch + 1) * CB],
                            start=(gi == 0),
                            stop=(final_stop and gi == 3))
                else:
                    nc.tensor.matmul(G[:, 0:4 * CB], W["B2"][:, 0:128],
                                     W["ONESB"][:, 0:4 * CB],
                                     start=True, stop=False)
                    for gi in range(4):
                        nc.tensor.matmul(
                            G[:, gi * CB:(gi + 1) * CB],
                            W["WX2"][:, gi * 128:(gi + 1) * 128],
                            h1[:, k * GB + ch * CB:k * GB + (ch + 1) * CB],
                            start=False,
                            stop=(final_stop and gi == 3))
                return G

            def scan_body(iv=None):
                for layer in (1, 2):
                    WL = W["WL1"] if layer == 1 else W["WL2"]
                    tca = tc1 if layer == 1 else tc2
                    qa = q1 if layer == 1 else q2

                    # prologue: x DMAs (prefetch distance 2) + step-0 input
                    # projections
                    if layer == 1:
                        xq = [xp.tile([20, GB], F16, tag="x", name="x")
                              for _ in (0, 1)]
                        nc.sync.dma_start(xq[0][:], d["xs"][0])
                        if n_steps > 1:
                            nc.sync.dma_start(xq[1][:], d["xs"][1])
                    else:
                        xq = [None, None]
                    Tcur = []
                    for ch in (0, 1):
                        t = new_CT(ch)
                        if layer == 1:
                            nc.vector.tensor_scalar_mul(
                                t[:, 0:CB],
                                t0b[:, ch * CB:(ch + 1) * CB], 2.0)
                        else:
                            nc.vector.memset(t[:, 0:CB], 0.0)
                        Tcur.append(t)
                    Gcur = [emit_inputs(layer, 0, ch, xq[0]) for ch in (0, 1)]
                    Gnext = [None, None]

                    for k in range(n_steps):
                        if layer == 1:
                            rhs_idx, out_idx = n_steps - k, n_steps - 1 - k
                        else:
                            rhs_idx, out_idx = k, k + 1
                        # prefetch x for step k+2
                        if layer == 1 and k + 2 < n_steps:
                            xfut = xp.tile([20, GB], F16, tag="x")
                            nc.sync.dma_start(xfut[:], d["xs"][k + 2])
                        else:
                            xfut = None

                        for ch in (0, 1):
                            # input projections for step k+1 (off critical
                            # path, fresh PSUM bank)
                            if k + 1 < n_steps:
                                Gnext[ch] = emit_inputs(layer, k + 1, ch,
                                                        xq[1])
                            G = Gcur[ch]
                            # recurrent matmuls: tc-side then q-side (same
                            # weights -> one LDWEIGHTS per gate)
                            rtc = tca[:, rhs_idx * GB + ch * CB:
                                      rhs_idx * GB + (ch + 1) * CB]
                            have_q = k > 0
                            for gi in range(4):
                                nc.tensor.matmul(
                                    G[:, gi * CB:(gi + 1) * CB],
                                    WL[:, gi * 128:(gi + 1) * 128],
                                    rtc, start=False,
                                    stop=(not have_q and gi == 3))
                            if have_q:
                                rq = qa[:, rhs_idx * GB + ch * CB:
                                        rhs_idx * GB + (ch + 1) * CB]
                                for gi in range(4):
                                    nc.tensor.matmul(
                                        G[:, gi * CB:(gi + 1) * CB],
                                        WL[:, gi * 128:(gi + 1) * 128],
                                        rq, start=False, stop=(gi == 3))
                            CT = Tcur[ch]
                            if split_o:
                                nc.scalar.activation(CT[:, CB:4 * CB],
                                                     G[:, 0:3 * CB], TANH)
                                nc.scalar.activation(CT[:, 4 * CB:5 * CB],
                                                     G[:, 3 * CB:4 * CB],
                                                     TANH)
                            else:
                                nc.scalar.activation(CT[:, CB:5 * CB], G[:],
                                                     TANH)
                            CTn = new_CT(ch)
                            UV = work.tile([128, 2 * CB], F16, tag="UV")
                            # u = (tf+1)*c*,  v = (ti+1)*tg (fused)
                            nc.vector.scalar_tensor_tensor(
                                UV[:], CT[:, 2 * CB:4 * CB], 1.0,
                                CT[:, 0:2 * CB], ADD, MULT)
                            # c* = 0.5*u + v -> next CT's c-slot
                            nc.vector.scalar_tensor_tensor(
                                CTn[:, 0:CB], UV[:, 0:CB], 0.5,
                                UV[:, CB:2 * CB], MULT, ADD)
                            # tc = tanh(0.5 c*) straight into the state tile
                            tdst = tca[:, out_idx * GB + ch * CB:
                                       out_idx * GB + (ch + 1) * CB]
                            nc.scalar.activation(tdst, CTn[:, 0:CB],
                                                 TANH, scale=0.5)
                            # q = to * tc (fast 2x tensor_tensor)
                            qdst = qa[:, out_idx * GB + ch * CB:
                                      out_idx * GB + (ch + 1) * CB]
                            nc.vector.tensor_tensor(
                                qdst, CT[:, 4 * CB:5 * CB], tdst, MULT)
                            if layer == 1:
                                # materialize h1* = (to+1)*tc for layer 2's
                                # input projections (off critical path)
                                nc.vector.scalar_tensor_tensor(
                                    h1[:, out_idx * GB + ch * CB:
                                       out_idx * GB + (ch + 1) * CB],
                                    CT[:, 4 * CB:5 * CB], 1.0, tdst,
                                    ADD, MULT)
                            Tcur[ch] = CTn
                            Gcur[ch] = Gnext[ch]
                        # roll the x queue
                        if layer == 1:
                            xq = [xq[1], xfut]

            if reps:
                with tc.For_i(0, reps, 1) as iv:
                    scan_body(iv)
            else:
                scan_body()

            # ---- output projection: y = (tc2+q2) @ (w_out/2).T + b_out ----
            YCH = 1024                       # free elems per chunk
            total = n_steps * GB
            nch = total // YCH
            for ci in range(nch):
                py = psY.tile([128, YCH], F32, tag="PY")
                for g in range(NG):
                    for j in range(YCH // 512):
                        off = GB + ci * YCH + j * 512
                        nc.tensor.matmul(py[32 * g:32 * g + 1,
                                            j * 512:(j + 1) * 512],
                                         W["WOUT"][32 * g:32 * g + 32, 0:1],
                                         tc2[32 * g:32 * g + 32,
                                             off:off + 512],
                                         start=True, stop=False,
                                         tile_position=(32 * g, 32 * g))
                        nc.tensor.matmul(py[32 * g:32 * g + 1,
                                            j * 512:(j + 1) * 512],
                                         W["WOUT"][32 * g:32 * g + 32, 0:1],
                                         q2[32 * g:32 * g + 32,
                                            off:off + 512],
                                         start=False, stop=True,
                                         tile_position=(32 * g, 32 * g))
                ysb = yp.tile([128, YCH], F32, tag="ysb")
                nc.scalar.activation(ysb[:], py[:], IDENT, bias=W["BOUT"][:, 0:1])
                for g in range(NG):
                    nc.sync.dma_start(y_out[g, ci * YCH:(ci + 1) * YCH],
                                      ysb[32 * g:32 * g + 1, :])
    nc.finalize()
    return nc


def kernel(**inputs):
    inputs = {k: np.asarray(v) for k, v in inputs.items()}
    if "nc" not in _CACHED:
        _CACHED["nc"] = build_program(S)
    nc = _CACHED["nc"]

    wts = _prep_weights(inputs)
    x = inputs["inputs_main"]          # [B, S, NX]
    sfc = inputs["inputs_sfc"]         # [B, NSFC]

    in_maps = []
    for c in range(NCORES):
        xs_c = x[c * BC:(c + 1) * BC]          # [BC, S, NX]
        sfc_c = sfc[c * BC:(c + 1) * BC]       # [BC, NSFC]
        # xs[s, 5g+q, r] = x[256g+r, S-1-s, q] for q<4; 1.0 for q=4
        xr = xs_c[:, ::-1, :]                  # time reversed
        xg = xr.reshape(NG, GB, S, NX).transpose(2, 0, 3, 1)  # [S, NG, NX, GB]
        xs_arr = np.ones((S, NG, 5, GB), np.float32)
        xs_arr[:, :, :NX, :] = xg
        xs_arr = xs_arr.reshape(S, 20, GB)
        sfcT = np.zeros((8, BC), np.float32)
        sfcT[:NSFC] = sfc_c.T
        m = {"xs": xs_arr.astype(np.float16), "sfcT": sfcT}
        m.update(wts)
        in_maps.append(m)

    res = run_bass_kernel_spmd(nc, in_maps, list(range(NCORES)))

    y = np.empty((B, S, NY), np.float32)
    for c in range(NCORES):
        yc = res.results[c]["y"]               # [NG, S*GB]
        yc = yc.reshape(NG, S, GB).transpose(0, 2, 1)   # [NG, GB, S]
        y[c * BC:(c + 1) * BC, :, 0] = yc.reshape(BC, S)
    return y


# revision 8
# speedup vs baseline: 1.0982x; 1.0982x over previous
"""Trainium2 Bass kernel for nn_BasicRNN (2-layer LSTM, H=32, S=64, B=8192).

Strategy: pure data parallel over 8 cores (1024 batch each). Per core the
batch is laid out in "T-layout" tiles [128 partitions = 4 groups x 32
features, 256 batch (free)]; the 256-batch free dim is split into three
chunks (88/84/84) whose dependency chains software-pipeline across the
engines (three phase-shifted chains keep ScalarE/DVE busy while each
chunk's serial recurrence closes).

The wall-clock is bound by the 128-step serial recurrence (64 steps x 2
layers); per chunk-step the critical cycle is
    h -> 4 recurrent matmuls -> tanh(gates) -> u,v -> c* -> tanh(c*) -> h
HW-measured notes (invisible in the cost-model sim): every stationary
weight switch costs ~100ns on the PE path, so input projections for step
k+1 are emitted at the TOP of step k, gate-major, with each loaded weight
serving all three chunks; x DMA prefetch distance is 2.

Sigmoids use the tanh identity sigmoid(x) = (1 + tanh(x/2))/2 with scale
factors folded into host-prepped weights; cell and hidden state are stored
doubled (c* = 2c, h* = 2h):
    u  = (tanh_f + 1) * c*          v = (tanh_i + 1) * tanh_g
    c* = 0.5*u + v                  h* = (tanh_o + 1) * tanh(0.5 * c*)
"""
import sys
sys.path.insert(0, '/opt/trn_rl_repo')

import numpy as np

import concourse.bacc as bacc
import concourse.tile as tile
from concourse import mybir
from concourse.bass_utils import run_bass_kernel_spmd

F32 = mybir.dt.float32
F16 = mybir.dt.float16
TANH = mybir.ActivationFunctionType.Tanh
IDENT = mybir.ActivationFunctionType.Identity
ADD = mybir.AluOpType.add
MULT = mybir.AluOpType.mult

B, S, NX, NSFC, H, NY = 8192, 64, 4, 5, 32, 1
NCORES = 8
BC = B // NCORES
NG = 4
GB = BC // NG
CHUNKS = [(0, 88), (88, 84), (172, 84)]
GATES = [("g", 2 * H, 1.0), ("f", H, 0.5), ("i", 0, 0.5), ("o", 3 * H, 0.5)]

def _prep_weights(inp):
    w = {}

    def blockdiag(wmat, scale_fio, scale_g):
        t = np.zeros((128, 512), np.float32)
        for gi, (nm, r0, _) in enumerate(GATES):
            s = scale_g if nm == "g" else scale_fio
            blk = (wmat[r0:r0 + H] * s).T.astype(np.float32)
            for g in range(NG):
                t[32 * g:32 * g + H,
                  128 * gi + 32 * g:128 * gi + 32 * g + 32] = blk
        return t.astype(np.float16)

    t = np.zeros((20, 512), np.float32)
    btot1 = inp["b_ih1"] + inp["b_hh1"]
    for gi, (nm, r0, trick) in enumerate(GATES):
        s = 1.0 if nm == "g" else 0.5
        blk = (inp["w_ih1"][r0:r0 + H] * s).T.astype(np.float32)
        for g in range(NG):
            c0 = 128 * gi + 32 * g
            t[5 * g:5 * g + NX, c0:c0 + 32] = blk
            t[5 * g + NX, c0:c0 + 32] = btot1[r0:r0 + H] * trick
    w["WX1"] = t.astype(np.float16)

    w["WL1"] = blockdiag(inp["w_hh1"], 0.25, 0.5)
    w["WX2"] = blockdiag(inp["w_ih2"], 0.25, 0.5)
    w["WL2"] = blockdiag(inp["w_hh2"], 0.25, 0.5)

    # layer-2 bias matmul: B2 [128, 128] as before; ONESB [128, 4, 256]:
    # onesb[32g+a, a, :] = 1.0 (gate-major blocks sliced per chunk).
    b2 = np.zeros((128, 128), np.float32)
    onesb = np.zeros((128, 4, 256), np.float32)
    btot2 = inp["b_ih2"] + inp["b_hh2"]
    for gi, (_, r0, trick) in enumerate(GATES):
        for g in range(NG):
            b2[32 * g + gi, 32 * g:32 * g + 32] = btot2[r0:r0 + H] * trick
            onesb[32 * g + gi, gi, :] = 1.0
    w["B2"] = b2.astype(np.float16)
    w["ONESB"] = onesb.astype(np.float16)

    ws = np.zeros((8, 64), np.float32)
    ws[:NSFC, 0:32] = inp["w_sfc1"].T
    ws[:NSFC, 32:64] = inp["w_sfc2"].T
    w["WSFC"] = ws
    bs = np.zeros((128, 2), np.float32)
    for g in range(NG):
        bs[32 * g:32 * g + 32, 0] = inp["b_sfc1"]
        bs[32 * g:32 * g + 32, 1] = inp["b_sfc2"]
    w["BSFC"] = bs

    wo = np.zeros((128, 1), np.float32)
    for g in range(NG):
        wo[32 * g:32 * g + 32, 0] = inp["w_out"][0] * 0.5
    w["WOUT"] = wo.astype(np.float16)
    w["BOUT"] = np.full((128, 1), float(inp["b_out"][0]), np.float32)
    return w



_CACHED = {}


def build_program(n_steps=S, trace_sim=False, reps=0):
    nc = bacc.Bacc()
    d = {}
    d["xs"] = nc.declare_dram_parameter("xs", [n_steps, 20, GB], F16,
                                        isOutput=False)
    d["sfcT"] = nc.declare_dram_parameter("sfcT", [8, BC], F32, isOutput=False)
    F16W = {"WX1", "WL1", "WX2", "WL2", "B2", "ONESB", "WOUT"}
    WSHAPES = [("WX1", [20, 512]), ("WL1", [128, 512]),
               ("WX2", [128, 512]), ("WL2", [128, 512]),
               ("B2", [128, 128]), ("ONESB", [128, 4, 256]),
               ("WSFC", [8, 64]), ("BSFC", [128, 2]),
               ("WOUT", [128, 1]), ("BOUT", [128, 1])]
    for nm, shape in WSHAPES:
        d[nm] = nc.declare_dram_parameter(nm, shape,
                                          F16 if nm in F16W else F32,
                                          isOutput=False)
    y_out = nc.declare_dram_parameter("y", [NG, n_steps * GB], F32,
                                      isOutput=True)

    NS1 = n_steps + 1
    NCH = len(CHUNKS)

    with tile.TileContext(nc, trace_sim=trace_sim) as tc:
        with tc.tile_pool(name="wpool", bufs=1) as wpool, \
             tc.tile_pool(name="big", bufs=1) as big, \
             tc.tile_pool(name="work", bufs=3) as work, \
             tc.tile_pool(name="xp", bufs=3) as xp, \
             tc.tile_pool(name="yp", bufs=2) as yp, \
             tc.tile_pool(name="psA", bufs=2, space="PSUM") as psA, \
             tc.tile_pool(name="psY", bufs=1, space="PSUM") as psY:

            W = {}
            for nm, shape in WSHAPES:
                t = wpool.tile(shape, F16 if nm in F16W else F32, tag=nm)
                nc.sync.dma_start(t[:], d[nm][:])
                W[nm] = t
            sfcT = wpool.tile([8, BC], F32, tag="sfcT")
            nc.sync.dma_start(sfcT[:], d["sfcT"][:])

            h1_all = big.tile([128, NS1 * GB], F16, tag="h1_all")
            h2_all = big.tile([128, NS1 * GB], F16, tag="h2_all")

            def new_CT(ch, w):
                return work.tile([128, 5, w], F16, tag=f"CT{ch}",
                                 name=f"CT{ch}")

            ph = psA.tile([128, 2 * GB], F32, tag="G0")
            for g in range(NG):
                nc.tensor.matmul(ph[32 * g:32 * g + 32, 0:GB],
                                 W["WSFC"][0:NSFC, 0:32],
                                 sfcT[0:NSFC, GB * g:GB * (g + 1)],
                                 start=True, stop=True,
                                 tile_position=(0, 32 * g))
                nc.tensor.matmul(ph[32 * g:32 * g + 32, GB:2 * GB],
                                 W["WSFC"][0:NSFC, 32:64],
                                 sfcT[0:NSFC, GB * g:GB * (g + 1)],
                                 start=True, stop=True,
                                 tile_position=(0, 32 * g))
            t0 = work.tile([128, GB], F32, tag="t0")
            nc.scalar.activation(t0[:], ph[:, 0:GB], TANH, bias=W["BSFC"][:, 0:1])
            nc.vector.tensor_scalar_mul(
                h1_all[:, n_steps * GB:(n_steps + 1) * GB], t0[:], 2.0)
            t0b = work.tile([128, GB], F32, tag="t0")
            nc.scalar.activation(t0b[:], ph[:, GB:2 * GB], TANH,
                                 bias=W["BSFC"][:, 1:2])
            nc.vector.memset(h2_all[:, 0:GB], 0.0)

            def emit_inputs_all(layer, k, xstep):
                """Gate-major input projections for ALL chunks of step k."""
                Gs = [psA.tile([128, 4, w], F32, tag=f"G{ch}", name=f"G{ch}")
                      for ch, (off, w) in enumerate(CHUNKS)]
                if layer == 1:
                    for gi in range(4):
                        for ch, (off, w) in enumerate(CHUNKS):
                            nc.tensor.matmul(
                                Gs[ch][:, gi, :],
                                W["WX1"][0:20, gi * 128:(gi + 1) * 128],
                                xstep[0:20, off:off + w],
                                start=(gi == 0), stop=False)
                else:
                    for ch, (off, w) in enumerate(CHUNKS):
                        nc.tensor.matmul(Gs[ch][:, :, :], W["B2"][:, 0:128],
                                         W["ONESB"][:, :, off:off + w],
                                         start=True, stop=False)
                    for gi in range(4):
                        for ch, (off, w) in enumerate(CHUNKS):
                            nc.tensor.matmul(
                                Gs[ch][:, gi, :],
                                W["WX2"][:, gi * 128:(gi + 1) * 128],
                                h1_all[:, k * GB + off:k * GB + off + w],
                                start=False, stop=False)
                return Gs

            def scan_body(iv=None):
                for layer in (1, 2):
                    WL = W["WL1"] if layer == 1 else W["WL2"]
                    hall = h1_all if layer == 1 else h2_all

                    xq = [None, None]
                    if layer == 1:
                        xq = [xp.tile([20, GB], F16, tag="x", name="x")
                              for _ in (0, 1)]
                        nc.sync.dma_start(xq[0][:], d["xs"][0])
                        if n_steps > 1:
                            nc.sync.dma_start(xq[1][:], d["xs"][1])
                    Tcur = []
                    for ch, (off, w) in enumerate(CHUNKS):
                        t = new_CT(ch, w)
                        if layer == 1:
                            nc.vector.tensor_scalar_mul(
                                t[:, 0, :], t0b[:, off:off + w], 2.0)
                        else:
                            nc.vector.memset(t[:, 0, :], 0.0)
                        Tcur.append(t)
                    Gcur = emit_inputs_all(layer, 0, xq[0])
                    Gnext = [None] * NCH

                    for k in range(n_steps):
                        if layer == 1:
                            rhs_idx, out_idx = n_steps - k, n_steps - 1 - k
                        else:
                            rhs_idx, out_idx = k, k + 1
                        if layer == 1 and k + 2 < n_steps:
                            xfut = xp.tile([20, GB], F16, tag="x")
                            nc.sync.dma_start(xfut[:], d["xs"][k + 2])
                        else:
                            xfut = None
                        if k + 1 < n_steps:
                            Gnext = emit_inputs_all(layer, k + 1, xq[1])

                        for ch in range(NCH):
                            off, w = CHUNKS[ch]
                            G = Gcur[ch]
                            rhs = hall[:, rhs_idx * GB + off:
                                       rhs_idx * GB + off + w]
                            for gi in range(4):
                                nc.tensor.matmul(
                                    G[:, gi, :],
                                    WL[:, gi * 128:(gi + 1) * 128],
                                    rhs, start=False, stop=(gi == 3))
                            CT = Tcur[ch]
                            nc.scalar.activation(CT[:, 1:5, :], G[:, :, :],
                                                 TANH)
                            CTn = new_CT(ch, w)
                            UV = work.tile([128, 2, w], F16, tag="UV")
                            nc.vector.scalar_tensor_tensor(
                                UV[:], CT[:, 2:4, :], 1.0,
                                CT[:, 0:2, :], ADD, MULT)
                            nc.vector.scalar_tensor_tensor(
                                CTn[:, 0, :], UV[:, 0, :], 0.5,
                                UV[:, 1, :], MULT, ADD)
                            TC = work.tile([128, w], F16, tag="TC")
                            nc.scalar.activation(TC[:], CTn[:, 0, :],
                                                 TANH, scale=0.5)
                            nc.vector.scalar_tensor_tensor(
                                hall[:, out_idx * GB + off:
                                     out_idx * GB + off + w],
                                CT[:, 4, :], 1.0, TC[:], ADD, MULT)
                            Tcur[ch] = CTn
                            Gcur[ch] = Gnext[ch]
                        if layer == 1:
                            xq = [xq[1], xfut]

            if reps:
                with tc.For_i(0, reps, 1) as iv:
                    scan_body(iv)
            else:
                scan_body()

            YCH = 1024
            total = n_steps * GB
            nch = total // YCH
            for ci in range(nch):
                py = psY.tile([128, YCH], F32, tag="PY")
                for g in range(NG):
                    for j in range(YCH // 512):
                        off = GB + ci * YCH + j * 512
                        nc.tensor.matmul(py[32 * g:32 * g + 1,
                                            j * 512:(j + 1) * 512],
                                         W["WOUT"][32 * g:32 * g + 32, 0:1],
                                         h2_all[32 * g:32 * g + 32,
                                                off:off + 512],
                                         start=True, stop=True,
                                         tile_position=(32 * g, 32 * g))
                ysb = yp.tile([128, YCH], F32, tag="ysb")
                nc.scalar.activation(ysb[:], py[:], IDENT, bias=W["BOUT"][:, 0:1])
                for g in range(NG):
                    nc.sync.dma_start(y_out[g, ci * YCH:(ci + 1) * YCH],
                                      ysb[32 * g:32 * g + 1, :])
    nc.finalize()
    return nc


def kernel(**inputs):
    inputs = {k: np.asarray(v) for k, v in inputs.items()}
    if "nc" not in _CACHED:
        _CACHED["nc"] = build_program(S)
    nc = _CACHED["nc"]
    wts = _prep_weights(inputs)
    x = inputs["inputs_main"]
    sfc = inputs["inputs_sfc"]
    in_maps = []
    for c in range(NCORES):
        xs_c = x[c * BC:(c + 1) * BC]
        sfc_c = sfc[c * BC:(c + 1) * BC]
        xr = xs_c[:, ::-1, :]
        xg = xr.reshape(NG, GB, S, NX).transpose(2, 0, 3, 1)
        xs_arr = np.ones((S, NG, 5, GB), np.float32)
        xs_arr[:, :, :NX, :] = xg
        xs_arr = xs_arr.reshape(S, 20, GB)
        sfcT = np.zeros((8, BC), np.float32)
        sfcT[:NSFC] = sfc_c.T
        m = {"xs": xs_arr.astype(np.float16), "sfcT": sfcT}
        m.update(wts)
        in_maps.append(m)
    res = run_bass_kernel_spmd(nc, in_maps, list(range(NCORES)))
    y = np.empty((B, S, NY), np.float32)
    for c in range(NCORES):
        yc = res.results[c]["y"]
        yc = yc.reshape(NG, S, GB).transpose(0, 2, 1)
        y[c * BC:(c + 1) * BC, :, 0] = yc.reshape(BC, S)
    return y


# revision 10
# speedup vs baseline: 1.1204x; 1.0202x over previous
"""Trainium2 Bass kernel for nn_BasicRNN (2-layer LSTM, H=32, S=64, B=8192).

Strategy: pure data parallel over 8 cores (1024 batch each). Per core the
batch is laid out in "T-layout" tiles [128 partitions = 4 groups x 32
features, 256 batch (free)]; the 256-batch free dim is split into four
64-wide chunks whose dependency chains software-pipeline across the
engines (four phase-shifted chains keep ScalarE/DVE busy while each
chunk's serial recurrence closes; this sits right at the ScalarE
busy-time floor of ~2.55us per step).

The wall-clock is bound by the 128-step serial recurrence (64 steps x 2
layers); per chunk-step the critical cycle is
    h -> 4 recurrent matmuls -> tanh(gates) -> u,v -> c* -> tanh(c*) -> h
HW-measured notes (invisible in the cost-model sim): every stationary
weight switch costs ~100ns on the PE path, so input projections for step
k+1 are emitted at the TOP of step k, gate-major, with each loaded weight
serving all four chunks; x DMA prefetch distance is 2.

Sigmoids use the tanh identity sigmoid(x) = (1 + tanh(x/2))/2 with scale
factors folded into host-prepped weights; cell and hidden state are stored
doubled (c* = 2c, h* = 2h):
    u  = (tanh_f + 1) * c*          v = (tanh_i + 1) * tanh_g
    c* = 0.5*u + v                  h* = (tanh_o + 1) * tanh(0.5 * c*)
"""
import sys
sys.path.insert(0, '/opt/trn_rl_repo')

import numpy as np

import concourse.bacc as bacc
import concourse.tile as tile
from concourse import mybir
from concourse.bass_utils import run_bass_kernel_spmd

F32 = mybir.dt.float32
F16 = mybir.dt.float16
TANH = mybir.ActivationFunctionType.Tanh
IDENT = mybir.ActivationFunctionType.Identity
ADD = mybir.AluOpType.add
MULT = mybir.AluOpType.mult

B, S, NX, NSFC, H, NY = 8192, 64, 4, 5, 32, 1
NCORES = 8
BC = B // NCORES
NG = 4
GB = BC // NG
CHUNKS = [(0, 64), (64, 64), (128, 64), (192, 64)]
GATES = [("g", 2 * H, 1.0), ("f", H, 0.5), ("i", 0, 0.5), ("o", 3 * H, 0.5)]

def _prep_weights(inp):
    w = {}

    def blockdiag(wmat, scale_fio, scale_g):
        t = np.zeros((128, 512), np.float32)
        for gi, (nm, r0, _) in enumerate(GATES):
            s = scale_g if nm == "g" else scale_fio
            blk = (wmat[r0:r0 + H] * s).T.astype(np.float32)
            for g in range(NG):
                t[32 * g:32 * g + H,
                  128 * gi + 32 * g:128 * gi + 32 * g + 32] = blk
        return t.astype(np.float16)

    t = np.zeros((20, 512), np.float32)
    btot1 = inp["b_ih1"] + inp["b_hh1"]
    for gi, (nm, r0, trick) in enumerate(GATES):
        s = 1.0 if nm == "g" else 0.5
        blk = (inp["w_ih1"][r0:r0 + H] * s).T.astype(np.float32)
        for g in range(NG):
            c0 = 128 * gi + 32 * g
            t[5 * g:5 * g + NX, c0:c0 + 32] = blk
            t[5 * g + NX, c0:c0 + 32] = btot1[r0:r0 + H] * trick
    w["WX1"] = t.astype(np.float16)

    w["WL1"] = blockdiag(inp["w_hh1"], 0.25, 0.5)
    w["WX2"] = blockdiag(inp["w_ih2"], 0.25, 0.5)
    w["WL2"] = blockdiag(inp["w_hh2"], 0.25, 0.5)

    # layer-2 bias matmul: B2 [128, 128] as before; ONESB [128, 4, 256]:
    # onesb[32g+a, a, :] = 1.0 (gate-major blocks sliced per chunk).
    b2 = np.zeros((128, 128), np.float32)
    onesb = np.zeros((128, 4, 256), np.float32)
    btot2 = inp["b_ih2"] + inp["b_hh2"]
    for gi, (_, r0, trick) in enumerate(GATES):
        for g in range(NG):
            b2[32 * g + gi, 32 * g:32 * g + 32] = btot2[r0:r0 + H] * trick
            onesb[32 * g + gi, gi, :] = 1.0
    w["B2"] = b2.astype(np.float16)
    w["ONESB"] = onesb.astype(np.float16)

    ws = np.zeros((8, 64), np.float32)
    ws[:NSFC, 0:32] = inp["w_sfc1"].T
    ws[:NSFC, 32:64] = inp["w_sfc2"].T
    w["WSFC"] = ws
    bs = np.zeros((128, 2), np.float32)
    for g in range(NG):
        bs[32 * g:32 * g + 32, 0] = inp["b_sfc1"]
        bs[32 * g:32 * g + 32, 1] = inp["b_sfc2"]
    w["BSFC"] = bs

    wo = np.zeros((128, 1), np.float32)
    for g in range(NG):
        wo[32 * g:32 * g + 32, 0] = inp["w_out"][0] * 0.5
    w["WOUT"] = wo.astype(np.float16)
    w["BOUT"] = np.full((128, 1), float(inp["b_out"][0]), np.float32)
    return w



_CACHED = {}


def build_program(n_steps=S, trace_sim=False, reps=0, split_o=False):
    split_o = split_o in (True, 'True')
    nc = bacc.Bacc()
    d = {}
    d["xs"] = nc.declare_dram_parameter("xs", [n_steps, 20, GB], F16,
                                        isOutput=False)
    d["sfcT"] = nc.declare_dram_parameter("sfcT", [8, BC], F32, isOutput=False)
    F16W = {"WX1", "WL1", "WX2", "WL2", "B2", "ONESB", "WOUT"}
    WSHAPES = [("WX1", [20, 512]), ("WL1", [128, 512]),
               ("WX2", [128, 512]), ("WL2", [128, 512]),
               ("B2", [128, 128]), ("ONESB", [128, 4, 256]),
               ("WSFC", [8, 64]), ("BSFC", [128, 2]),
               ("WOUT", [128, 1]), ("BOUT", [128, 1])]
    for nm, shape in WSHAPES:
        d[nm] = nc.declare_dram_parameter(nm, shape,
                                          F16 if nm in F16W else F32,
                                          isOutput=False)
    y_out = nc.declare_dram_parameter("y", [NG, n_steps * GB], F32,
                                      isOutput=True)

    NS1 = n_steps + 1
    NCH = len(CHUNKS)

    with tile.TileContext(nc, trace_sim=trace_sim) as tc:
        with tc.tile_pool(name="wpool", bufs=1) as wpool, \
             tc.tile_pool(name="big", bufs=1) as big, \
             tc.tile_pool(name="work", bufs=3) as work, \
             tc.tile_pool(name="xp", bufs=3) as xp, \
             tc.tile_pool(name="yp", bufs=2) as yp, \
             tc.tile_pool(name="psA", bufs=2, space="PSUM") as psA:

            W = {}
            for nm, shape in WSHAPES:
                t = wpool.tile(shape, F16 if nm in F16W else F32, tag=nm)
                nc.sync.dma_start(t[:], d[nm][:])
                W[nm] = t
            sfcT = wpool.tile([8, BC], F32, tag="sfcT")
            nc.sync.dma_start(sfcT[:], d["sfcT"][:])

            h1_all = big.tile([128, NS1 * GB], F16, tag="h1_all")
            h2_all = big.tile([128, NS1 * GB], F16, tag="h2_all")

            def new_CT(ch, w):
                return work.tile([128, 5, w], F16, tag=f"CT{ch}",
                                 name=f"CT{ch}")

            ph = psA.tile([128, 2 * GB], F32, tag="G0")
            for g in range(NG):
                nc.tensor.matmul(ph[32 * g:32 * g + 32, 0:GB],
                                 W["WSFC"][0:NSFC, 0:32],
                                 sfcT[0:NSFC, GB * g:GB * (g + 1)],
                                 start=True, stop=True,
                                 tile_position=(0, 32 * g))
                nc.tensor.matmul(ph[32 * g:32 * g + 32, GB:2 * GB],
                                 W["WSFC"][0:NSFC, 32:64],
                                 sfcT[0:NSFC, GB * g:GB * (g + 1)],
                                 start=True, stop=True,
                                 tile_position=(0, 32 * g))
            t0 = work.tile([128, GB], F32, tag="t0")
            nc.scalar.activation(t0[:], ph[:, 0:GB], TANH, bias=W["BSFC"][:, 0:1])
            nc.vector.tensor_scalar_mul(
                h1_all[:, n_steps * GB:(n_steps + 1) * GB], t0[:], 2.0)
            t0b = work.tile([128, GB], F32, tag="t0")
            nc.scalar.activation(t0b[:], ph[:, GB:2 * GB], TANH,
                                 bias=W["BSFC"][:, 1:2])
            nc.vector.memset(h2_all[:, 0:GB], 0.0)

            def emit_inputs_all(layer, k, xstep):
                """Gate-major input projections for ALL chunks of step k."""
                Gs = [psA.tile([128, 4, w], F32, tag=f"G{ch}", name=f"G{ch}")
                      for ch, (off, w) in enumerate(CHUNKS)]
                if layer == 1:
                    for gi in range(4):
                        for ch, (off, w) in enumerate(CHUNKS):
                            nc.tensor.matmul(
                                Gs[ch][:, gi, :],
                                W["WX1"][0:20, gi * 128:(gi + 1) * 128],
                                xstep[0:20, off:off + w],
                                start=(gi == 0), stop=False)
                else:
                    for ch, (off, w) in enumerate(CHUNKS):
                        nc.tensor.matmul(Gs[ch][:, :, :], W["B2"][:, 0:128],
                                         W["ONESB"][:, :, off:off + w],
                                         start=True, stop=False)
                    for gi in range(4):
                        for ch, (off, w) in enumerate(CHUNKS):
                            nc.tensor.matmul(
                                Gs[ch][:, gi, :],
                                W["WX2"][:, gi * 128:(gi + 1) * 128],
                                h1_all[:, k * GB + off:k * GB + off + w],
                                start=False, stop=False)
                return Gs

            def scan_body(iv=None):
                for layer in (1, 2):
                    WL = W["WL1"] if layer == 1 else W["WL2"]
                    hall = h1_all if layer == 1 else h2_all

                    xq = [None, None]
                    if layer == 1:
                        xq = [xp.tile([20, GB], F16, tag="x", name="x")
                              for _ in (0, 1)]
                        nc.sync.dma_start(xq[0][:], d["xs"][0])
                        if n_steps > 1:
                            nc.sync.dma_start(xq[1][:], d["xs"][1])
                    Tcur = []
                    for ch, (off, w) in enumerate(CHUNKS):
                        t = new_CT(ch, w)
                        if layer == 1:
                            nc.vector.tensor_scalar_mul(
                                t[:, 0, :], t0b[:, off:off + w], 2.0)
                        else:
                            nc.vector.memset(t[:, 0, :], 0.0)
                        Tcur.append(t)
                    Gcur = emit_inputs_all(layer, 0, xq[0])
                    Gnext = [None] * NCH

                    for k in range(n_steps):
                        if layer == 1:
                            rhs_idx, out_idx = n_steps - k, n_steps - 1 - k
                        else:
                            rhs_idx, out_idx = k, k + 1
                        if layer == 1 and k + 2 < n_steps:
                            xfut = xp.tile([20, GB], F16, tag="x")
                            nc.sync.dma_start(xfut[:], d["xs"][k + 2])
                        else:
                            xfut = None
                        if k + 1 < n_steps:
                            Gnext = emit_inputs_all(layer, k + 1, xq[1])

                        for ch in range(NCH):
                            off, w = CHUNKS[ch]
                            G = Gcur[ch]
                            rhs = hall[:, rhs_idx * GB + off:
                                       rhs_idx * GB + off + w]
                            for gi in range(4):
                                nc.tensor.matmul(
                                    G[:, gi, :],
                                    WL[:, gi * 128:(gi + 1) * 128],
                                    rhs, start=False, stop=(gi == 3))
                            CT = Tcur[ch]
                            if split_o:
                                nc.scalar.activation(CT[:, 1:4, :],
                                                     G[:, 0:3, :], TANH)
                                nc.scalar.activation(CT[:, 4, :],
                                                     G[:, 3, :], TANH)
                            else:
                                nc.scalar.activation(CT[:, 1:5, :],
                                                     G[:, :, :], TANH)
                            CTn = new_CT(ch, w)
                            UV = work.tile([128, 2, w], F16, tag="UV")
                            nc.vector.scalar_tensor_tensor(
                                UV[:], CT[:, 2:4, :], 1.0,
                                CT[:, 0:2, :], ADD, MULT)
                            nc.vector.scalar_tensor_tensor(
                                CTn[:, 0, :], UV[:, 0, :], 0.5,
                                UV[:, 1, :], MULT, ADD)
                            TC = work.tile([128, w], F16, tag="TC")
                            nc.scalar.activation(TC[:], CTn[:, 0, :],
                                                 TANH, scale=0.5)
                            nc.vector.scalar_tensor_tensor(
                                hall[:, out_idx * GB + off:
                                     out_idx * GB + off + w],
                                CT[:, 4, :], 1.0, TC[:], ADD, MULT)
                            Tcur[ch] = CTn
                            Gcur[ch] = Gnext[ch]
                        if layer == 1:
                            xq = [xq[1], xfut]

            if reps:
                with tc.For_i(0, reps, 1) as iv:
                    scan_body(iv)
            else:
                scan_body()

            YCH = 512
            total = n_steps * GB
            nch = total // YCH
            for ci in range(nch):
                py = psA.tile([128, YCH], F32, tag="G0", name="py")
                for g in range(NG):
                    for j in range(YCH // 512):
                        off = GB + ci * YCH + j * 512
                        nc.tensor.matmul(py[32 * g:32 * g + 1,
                                            j * 512:(j + 1) * 512],
                                         W["WOUT"][32 * g:32 * g + 32, 0:1],
                                         h2_all[32 * g:32 * g + 32,
                                                off:off + 512],
                                         start=True, stop=True,
                                         tile_position=(32 * g, 32 * g))
                ysb = yp.tile([128, YCH], F32, tag="ysb")
                nc.scalar.activation(ysb[:], py[:], IDENT, bias=W["BOUT"][:, 0:1])
                for g in range(NG):
                    nc.sync.dma_start(y_out[g, ci * YCH:(ci + 1) * YCH],
                                      ysb[32 * g:32 * g + 1, :])
    nc.finalize()
    return nc


def kernel(**inputs):
    inputs = {k: np.asarray(v) for k, v in inputs.items()}
    if "nc" not in _CACHED:
        _CACHED["nc"] = build_program(S)
    nc = _CACHED["nc"]
    wts = _prep_weights(inputs)
    x = inputs["inputs_main"]
    sfc = inputs["inputs_sfc"]
    in_maps = []
    for c in range(NCORES):
        xs_c = x[c * BC:(c + 1) * BC]
        sfc_c = sfc[c * BC:(c + 1) * BC]
        xr = xs_c[:, ::-1, :]
        xg = xr.reshape(NG, GB, S, NX).transpose(2, 0, 3, 1)
        xs_arr = np.ones((S, NG, 5, GB), np.float32)
        xs_arr[:, :, :NX, :] = xg
        xs_arr = xs_arr.reshape(S, 20, GB)
        sfcT = np.zeros((8, BC), np.float32)
        sfcT[:NSFC] = sfc_c.T
        m = {"xs": xs_arr.astype(np.float16), "sfcT": sfcT}
        m.update(wts)
        in_maps.append(m)
    res = run_bass_kernel_spmd(nc, in_maps, list(range(NCORES)))
    y = np.empty((B, S, NY), np.float32)
    for c in range(NCORES):
        yc = res.results[c]["y"]
        yc = yc.reshape(NG, S, GB).transpose(0, 2, 1)
        y[c * BC:(c + 1) * BC, :, 0] = yc.reshape(BC, S)
    return y
